# revision 18
# baseline (speedup 1.0000x reference)
"""Hybrid-sharded variant: 4 batch quarters x 2 sequence halves.

Per core: 1024 batches x 21 owned positions. Half 1 runs the sequence
REVERSED (host mirrors x slices, W taps, and un-reverses the output), which
makes the two halves' programs identical with NO zero halo slices at all:
every position j consumes x slices j-1..j+1; position 0 simply has no j-1
tap (that input is the zero pad on both halves, so the matmul is dropped).
The halves overlap at global position 20; each half computes/stores it for a
COMPLEMENTARY half of the batches (half 1's batch axis is rotated by 512 on
the host so the same uniform program covers disjoint global batches), and
x slice 21 - consumed only by position 20 - is likewise loaded half-width.

Bytes/core: x 21.5*1024*256 = 5.64MB + W 62 FxF chunks = 2.03MB + out
20.5*1024*256 = 5.37MB = 13.0MB -> ~36.2us busy at the 360GB/s DMA model.

Relu alternates between DVE and Act so the relu stream keeps pace with PE,
and stores issue early groups first so the DMA packs until the merged last
store group (positions 16..20) becomes ready right at its queue slot.
"""

import os
import sys

import numpy as np
import ml_dtypes

for _p in ("/opt/trn_rl_repo", "/root/.axon_site/_ro/trn_rl_repo"):
    if os.path.isdir(_p) and _p not in sys.path:
        sys.path.append(_p)

from contextlib import ExitStack

import concourse.mybir as mybir
import concourse.tile as tile
from concourse import bacc
from concourse.bass_utils import run_bass_kernel_spmd
from concourse.tile import add_dep_helper

S = 41
F = 128
WIN = 3
N_CORES = 8
B_FULL = 4096

SH = 2                    # sequence halves
PO = 21                   # owned positions per half (overlap at global 20)
XS = PO + 1               # x slices: position j consumes slices j-1..j+1
BQ = 4                    # batch quarters
NB = B_FULL // BQ         # 1024 batches per core
NBT = NB // 128           # 8 batch sub-tiles
GS = 4                    # positions per PSUM bank group

# x load groups (start, nslices): single-slice groups after [0..3] and at the
# end keep the in-order PE chains fed without stalling on a 4-slice transfer
_XGROUPS = [(0, 4), (4, 1), (5, 4), (9, 4), (13, 4), (17, 4), (21, 1)]

_nc_cache = {}


def _layout():
    """Matmul groups over local positions 0..PO-1. Entry (i, jmin, ncons):
    stationary x-slice i, consumer positions j in [jmin, jmin+ncons) with
    tap t = i - j + 1 (position 0's t=0 tap is the zero pad: dropped)."""
    out = []
    for j0 in range(0, PO, GS):
        n = min(GS, PO - j0)
        ents = []
        for i in range(max(0, j0 - 1), j0 + n + 1):
            jmin = max(j0, i - 1)
            jmax = min(j0 + n - 1, i + 1)
            if jmax >= jmin:
                ents.append((i, jmin, jmax - jmin + 1))
        out.append((j0, n, ents))
    return out


_LAYOUT = _layout()
_WCOLS = [sum(nc_ * F for _, _, nc_ in ents) for _, _, ents in _LAYOUT]
_WTOT = sum(_WCOLS)

# stage groups: matmul groups g4 (4 pos) and g5 (1 pos) share one stage tile
# so the tail store has >=512B dram runs and drains as one packed sequence
_STAGE_OF = [0, 1, 2, 3, 4, 4]
_STAGE_POS = [(0, 4), (4, 4), (8, 4), (12, 4), (16, 5)]

_N_WARMUP = 72  # dummy PE work ending right as the first real matmul's inputs land

_SLICE_LOC = {}
for _gi, (_s0, _ns) in enumerate(_XGROUPS):
    for _k in range(_ns):
        _SLICE_LOC[_s0 + _k] = (_gi, _k)


def _build(has_bias: bool):
    bf16 = mybir.dt.bfloat16
    f32 = mybir.dt.float32
    f16 = mybir.dt.float16
    relu = mybir.ActivationFunctionType.Relu
    nc = bacc.Bacc("TRN2", target_bir_lowering=False, debug=False)
    xT = nc.dram_tensor("xT", [XS, F, NB], bf16, kind="ExternalInput").ap()
    Wg = nc.dram_tensor("Wg", [F, _WTOT], bf16, kind="ExternalInput").ap()
    bias = (
        nc.dram_tensor("bias", [1, PO * F], bf16, kind="ExternalInput").ap()
        if has_bias
        else None
    )
    out = nc.dram_tensor("out", [NB, PO, F], f16, kind="ExternalOutput").ap()

    with tile.TileContext(nc) as tc:
        with ExitStack() as ctx:
            xpool = ctx.enter_context(tc.tile_pool(name="xT", bufs=len(_XGROUPS)))
            wpool = ctx.enter_context(tc.tile_pool(name="W", bufs=len(_LAYOUT)))
            ppool = ctx.enter_context(tc.tile_pool(name="ps", bufs=4, space="PSUM"))
            opool = ctx.enter_context(tc.tile_pool(name="stage", bufs=len(_STAGE_POS)))

            # loads interleaved x/W so each matmul group's inputs land just
            # ahead of the in-order PE chain that consumes them
            xt, wt, load_insts = [], [], []
            wcol0 = 0
            wg = 0

            def load_w():
                nonlocal wcol0, wg
                tw = wpool.tile([F, _WCOLS[wg]], bf16)
                li = nc.sync.dma_start(tw[:], Wg[:, wcol0 : wcol0 + _WCOLS[wg]])
                load_insts.append(li.ins)
                wt.append(tw)
                wcol0 += _WCOLS[wg]
                wg += 1

            for g, (s0, ns) in enumerate(_XGROUPS):
                # slice 21 feeds only position 20, which is stored for the
                # first NB/2 batches only - so load it half-width. It is
                # split into two pieces so the program has 24 SP DMAs: the
                # completion sems rotate over 8 handles, and with 24 the
                # final store lands on the LAST handle the exit sequence
                # checks, trimming one 50ns wait from the drain chain.
                nb = NB // 2 if s0 == 21 else NB
                tx = xpool.tile([F, ns * nb], bf16)
                nparts = 2 if s0 == 21 else 1
                for pc in range(nparts):
                    p0, p1 = pc * nb // nparts, (pc + 1) * nb // nparts
                    li = nc.sync.dma_start(
                        tx[:].rearrange("k (s b) -> k s b", b=nb)[:, :, p0:p1],
                        xT[s0 : s0 + ns].rearrange("s k b -> k s b")[:, :, p0:p1],
                    )
                    load_insts.append(li.ins)
                xt.append(tx)
                if g == 0 or g >= 2:
                    load_w()
            while wg < len(_LAYOUT):
                load_w()
            store_gate = load_insts[-7]

            if has_bias:
                bpool = ctx.enter_context(tc.tile_pool(name="bias", bufs=1))
                bias_sb = bpool.tile([1, PO * F], bf16)
                nc.scalar.dma_start(bias_sb[:], bias[:])
                ones = bpool.tile([1, F], bf16)
                nc.vector.memset(ones[:], 1.0)

            # PE p-state warmup: dummy zero matmuls keep PE continuously busy
            # from program start until the first x/W tiles land (~6.8us), so
            # every real matmul runs at the fully-ramped 2.4GHz cycle instead
            # of spending its first 3us at the mid p-state
            wmpool = ctx.enter_context(tc.tile_pool(name="warm", bufs=1))
            warm = wmpool.tile([128, F], bf16)
            nc.gpsimd.memset(warm[:], 0.0)
            warm_ps = ppool.tile([128, 2 * GS * F], f32, name="warm_ps", tag="ps")
            for _wi in range(_N_WARMUP):
                nc.tensor.matmul(
                    warm_ps[:, :F], lhsT=warm[:], rhs=warm[:], start=True, stop=True
                )

            out_r = out.rearrange("(t p) s f -> p t s f", p=128)

            stages = []
            for _sgi, (_, npos) in enumerate(_STAGE_POS):
                stage_t = opool.tile(
                    [128, NBT * npos * F], f16, tag="stage", name=f"stage{_sgi}"
                )
                stages.append(stage_t)
            relu_cnt = 0
            # g5 (the single overlap position) before g4: the merged tail
            # stage's relus then finish before its DMA queue slot arrives
            for g in (0, 1, 2, 3, 5, 4):
                s0, npos, ents = _LAYOUT[g]
                sg = _STAGE_OF[g]
                sg0, sgn = _STAGE_POS[sg]
                stage_c = stages[sg][:].rearrange("p (t c) -> p t c", t=NBT)
                d0 = (s0 - sg0) * F
                # two batch-subtiles share one 2-bank PSUM tile so a single
                # relu covers both (the relu stream paces the tail);
                # the overlap position (g5) only runs the stored batch half
                nbtp = NBT // 4 if s0 == 20 else NBT // 2
                for btp in range(nbtp):
                    ps = ppool.tile([128, 2 * GS * F], f32)
                    for half in range(2):
                        bt = btp * 2 + half
                        hb = half * GS * F
                        n_mm = len(ents) + (1 if has_bias else 0)
                        wcol = 0
                        for j, (si, jmin, ncons) in enumerate(ents):
                            gi, sub = _SLICE_LOC[si]
                            lhsT = xt[gi][
                                :, sub * NB + bt * 128 : sub * NB + (bt + 1) * 128
                            ]
                            c0 = hb + (jmin - s0) * F
                            nc.tensor.matmul(
                                ps[:, c0 : c0 + ncons * F],
                                lhsT=lhsT,
                                rhs=wt[g][:, wcol : wcol + ncons * F],
                                start=(j == 0),
                                stop=(j == n_mm - 1),
                            )
                            wcol += ncons * F
                        if has_bias:
                            nc.tensor.matmul(
                                ps[:, hb : hb + npos * F],
                                lhsT=ones[:],
                                rhs=bias_sb[:, s0 * F : (s0 + npos) * F],
                                start=False,
                                stop=True,
                            )
                    dst = stage_c[:, btp * 2 : btp * 2 + 2, d0 : d0 + npos * F]
                    src = ps[:].rearrange("p (h c) -> p h c", h=2)[:, :, : npos * F]
                    if relu_cnt % 2 == 0:
                        nc.vector.tensor_scalar_max(dst, src, 0.0)
                    else:
                        nc.scalar.activation(dst, src, relu)
                    relu_cnt += 1

            for sg, (sg0, sgn) in enumerate(_STAGE_POS):
                stage_v = stages[sg][:].rearrange(
                    "p (t s f) -> p t s f", t=NBT, f=F
                )
                for o in range(2):
                    hh = NBT // 2
                    # the merged tail stage stores position 20 only for the
                    # first batch half (the other half is owned by the peer)
                    ns_o = sgn if (sg < len(_STAGE_POS) - 1 or o == 0) else sgn - 1
                    st = nc.sync.dma_start(
                        out_r[:, o * hh : (o + 1) * hh, sg0 : sg0 + ns_o, :],
                        stage_v[:, o * hh : (o + 1) * hh, :ns_o, :],
                    )
                    add_dep_helper(
                        st.ins, store_gate, sync=True, reason="stores after loads"
                    )

    nc.compile()
    return nc


def _get_nc(has_bias: bool):
    if has_bias not in _nc_cache:
        _nc_cache[has_bias] = _build(has_bias)
    return _nc_cache[has_bias]


def _prep_in_maps(inputs: np.ndarray, W: np.ndarray, b: np.ndarray):
    has_bias = bool(np.any(b))
    xb = inputs.astype(ml_dtypes.bfloat16)
    Wb = W.astype(ml_dtypes.bfloat16)
    wgs, biases = [], []
    for h in range(SH):
        blocks = []
        for s0, npos, ents in _LAYOUT:
            for si, jmin, ncons in ents:
                for j in range(jmin, jmin + ncons):
                    t = si - j + 1  # device tap: slice si covers pos si-1..si+1
                    if h == 0:
                        blocks.append(Wb[j, t * F : (t + 1) * F, :])
                    else:
                        blocks.append(Wb[40 - j, (2 - t) * F : (3 - t) * F, :])
        wgs.append(np.ascontiguousarray(np.concatenate(blocks, axis=1)))
        assert wgs[-1].shape == (F, _WTOT)
        if has_bias:
            bh = np.empty((PO, F), ml_dtypes.bfloat16)
            for j in range(PO):
                bh[j] = b[j if h == 0 else 40 - j].astype(ml_dtypes.bfloat16)
            biases.append(np.ascontiguousarray(bh.reshape(1, PO * F)))

    in_maps = []
    for c in range(N_CORES):
        h, bp = divmod(c, BQ)
        xbc = xb[bp * NB : (bp + 1) * NB]  # [NB, S, F]
        if h == 0:
            xs = xbc[:, 0:XS, :]  # slice s = global s
        else:
            # rotate the batch axis so this half's first NB/2 local batches
            # are the global second half: the two halves then store the
            # overlap position (global 20) for complementary batches
            xbc = np.roll(xbc, -(NB // 2), axis=0)
            xs = xbc[:, 40 : 40 - XS : -1, :]  # slice s = global 40-s
        xs = np.ascontiguousarray(xs.transpose(1, 2, 0))  # [XS, F, NB]
        xs[XS - 1, :, NB // 2 :] = 0  # slice 21: only first NB/2 loaded
        m = {"xT": xs, "Wg": wgs[h]}
        if has_bias:
            m["bias"] = biases[h]
        in_maps.append(m)
    return in_maps, has_bias


def kernel(inputs: np.ndarray, W: np.ndarray, b: np.ndarray) -> np.ndarray:
    inputs = np.asarray(inputs)
    W = np.asarray(W)
    b = np.asarray(b)
    assert inputs.shape == (B_FULL, S, F), inputs.shape
    in_maps, has_bias = _prep_in_maps(inputs, W, b)
    nc = _get_nc(has_bias)
    res = run_bass_kernel_spmd(nc, in_maps, list(range(N_CORES)))
    out = np.empty((B_FULL, S, F), np.float32)
    for c in range(N_CORES):
        h, bp = divmod(c, BQ)
        r = res.results[c]["out"].astype(np.float32)  # [NB, PO, F]
        b0 = bp * NB
        if h == 0:
            out[b0 : b0 + NB, 0:20, :] = r[:, 0:20]
            out[b0 : b0 + NB // 2, 20, :] = r[: NB // 2, 20]
        else:
            # local j -> global 40-j; un-rotate the batch axis; position 20
            # is stored for local batches 0..NB/2 = global NB/2..NB
            ru = np.roll(r, NB // 2, axis=0)
            out[b0 : b0 + NB, PO:S, :] = ru[:, 19::-1, :]
            out[b0 + NB // 2 : b0 + NB, 20, :] = ru[NB // 2 :, 20]
    return out
